# revision 31
# baseline (speedup 1.0000x reference)
"""Trainium2 Bass kernel for nn_AdaptivePIDNetworkControllerV2.

Self-contained: kernel(**inputs) -> np.ndarray (B,) float32.

Algorithm
---------
Reference, per batch row b:
  ext = x[b,1];  s_c = MLP(x[b]) (32->16->8->C; relu, relu, linear)
  25-step PID scan per controller c over state
  (meas, dyn_sp, integral, prev_err); output mean_c(meas_25).

With Kd == 0 the scan reduces exactly to a 2-state recurrence in
(e = err, K = integral/DT), with u = e + gamma*K (meas = Kp*u):
  e'  = E''' + rho*u + sum_j c_j * max(u, P_j)
  K'  = clip(K + e', +-50)
where P_j = p1_j*s + p0_j (affine in s), E''' = ALPHA*ext + const, and
rho/c_j/p*_j are scalars derived from the dynamic-setpoint network
weights (relu(z_j) rewritten via max identities; signs folded).
Branches whose relu never clamps on a host subsample are folded into
rho exactly ("linear_u").

Layout: batch sharded 8 ways (pure data parallel).  Per core, rows go
through chunks of SUBS_PER_CHUNK*8192; the scan runs on fp16 tiles
(128, 512*SUBS_PER_CHUNK) with partition = 8*j + c and batch row
  row = chunk_base + 8192*sc + 2048*(j//4) + 512*d + 128*(j%4) + p
for column index 512*sc + 128*d + p.  The MLP runs feature-on-partition
via PE transposes and partition-stacked small matmuls; ext is extracted
with an extra "selector" matmul; the final mean over c is a matmul with
a block-diagonal (Kp_c/8) matrix.
"""

import numpy as np

B_TOTAL = 1048576
D_IN = 32
C = 8
STEPS = 25
ALPHA = 0.5
DT = 0.1
KCLIP = 5.0 / DT          # integral clip bound in K units

N_CORES = 8
SUB = 8192                # rows per MLP sub-chunk (fixed internal shape)



# ---------------------------------------------------------------------------
# host-side derivation
# ---------------------------------------------------------------------------

def _derive(inputs):
    f = lambda k: np.asarray(inputs[k], np.float64)
    P = dict(
        W1=f("W1"), b1=f("b1"), W2=f("W2"), b2=f("b2"),
        W3=f("W3"), b3=f("b3"),
    )
    Wd1, bd1 = f("Wd1"), f("bd1")
    Wd2, bd2 = f("Wd2").reshape(-1), f("bd2")
    Kp, Ki, Kd = f("Kp"), f("Ki"), f("Kd")

    P["kp"] = Kp
    P["gamma"] = Ki * DT / Kp
    P["delta"] = Kd / (DT * Kp)
    a_j = Wd1[:, 0]
    braw = Wd1[:, 1]
    w_j = (1 - ALPHA) * Wd2
    P["w0"] = (1 - ALPHA) * bd2[0]
    beta = braw[:, None] * Kp[None, :]           # (3, C)
    P["eta"] = np.sign(beta)
    P["cj"] = w_j[:, None] * np.abs(beta)        # (3, C)
    P["p1"] = -a_j[:, None] / beta               # (3, C)
    P["p0"] = -bd1[:, None] / beta               # (3, C)
    P["kd_nonzero"] = bool(np.any(np.abs(Kd) > 0))
    return P


def _host_mlp(x, P):
    h = np.maximum(x @ P["W1"].T + P["b1"], 0)
    h = np.maximum(h @ P["W2"].T + P["b2"], 0)
    return h @ P["W3"].T + P["b3"]


def _pick_branch_modes(inputs, P):
    """Decide per branch j: 'max' (keep), 'linear_u' (max(u,P)==u always),
    or 'linear_p' (==P always), from a host subsample of the recurrence."""
    if P["kd_nonzero"]:
        return ["max", "max", "max"]
    x = np.asarray(inputs["x"], np.float64)
    n = min(65536, x.shape[0])
    step = max(1, x.shape[0] // n)
    xs = x[::step][:n]
    s = _host_mlp(xs, P)
    ext = xs[:, 1]
    gamma = P["gamma"][None, :]
    kp = P["kp"][None, :]
    eta, cj, p1, p0 = P["eta"], P["cj"], P["p1"], P["p0"]
    Pj = p1[:, None, :] * s[None] + p0[:, None, :]
    rho = -kp - np.where(eta < 0, cj, 0.0).sum(0)[None, :]
    Em = ALPHA * ext[:, None] + P["w0"] \
        - (np.where(eta > 0, cj, 0.0)[:, None, :] * Pj).sum(0)
    def run(branch_fn):
        e = s.copy()
        K = np.clip(s, -KCLIP, KCLIP)
        for t in range(1, STEPS + 1):
            u = e + gamma * K
            if t == STEPS:
                break
            acc = rho * u + Em
            for j in range(3):
                acc = acc + cj[j][None, :] * branch_fn(j, u)
            e = acc
            K = np.clip(K + e, -KCLIP, KCLIP)
        return (kp * u).mean(axis=1)

    base = run(lambda j, u: np.maximum(u, Pj[j]))
    nrm = np.linalg.norm(base)
    modes = []
    for j in range(3):
        def lin_u(jj, u, j=j):
            return u if jj == j else np.maximum(u, Pj[jj])
        def lin_p(jj, u, j=j):
            return Pj[jj] if jj == j else np.maximum(u, Pj[jj])
        if np.linalg.norm(run(lin_u) - base) < 1e-4 * nrm:
            modes.append("linear_u")
        elif np.linalg.norm(run(lin_p) - base) < 1e-4 * nrm:
            modes.append("linear_p")
        else:
            modes.append("max")
    return modes


def _fold_constants(P, modes):
    """Per-branch contribution of c_j*relu-term to (rho, es1, es0), using
    hd_j = |beta_j|*relu(eta_j*(u - P_j)) identities:
      eta>0: c*relu(u-P)  = c*max(u,P) - c*P   (max mode)
                          = c*u - c*P          (linear_u)
                          = 0                  (linear_p)
      eta<0: c*relu(P-u)  = c*max(u,P) - c*u   (max mode)
                          = 0                  (linear_u)
                          = c*P - c*u          (linear_p)
    "max" mode additionally emits the c_j*max(u,P_j) op on device."""
    cj, eta, p1, p0 = P["cj"], P["eta"], P["p1"], P["p0"]
    rho = -P["kp"].copy()
    es1 = np.zeros(C)
    es0 = np.full(C, P["w0"])
    for j, m in enumerate(modes):
        pos = eta[j, 0] > 0
        if m == "max":
            if pos:
                es1 = es1 - cj[j] * p1[j]
                es0 = es0 - cj[j] * p0[j]
            else:
                rho = rho - cj[j]
        elif m == "linear_u":
            if pos:
                rho = rho + cj[j]
                es1 = es1 - cj[j] * p1[j]
                es0 = es0 - cj[j] * p0[j]
        elif m == "linear_p":
            if not pos:
                rho = rho - cj[j]
                es1 = es1 + cj[j] * p1[j]
                es0 = es0 + cj[j] * p0[j]
    return rho, es1, es0


def host_restructured(x, P, modes, clip_on=True):
    """Full-precision host implementation of exactly what the device
    computes (minus fp16 rounding); used for self-checks in test.py."""
    s = _host_mlp(x, P)
    ext = x[:, 1]
    gamma = P["gamma"][None, :]
    kp = P["kp"][None, :]
    eta, cj, p1, p0 = P["eta"], P["cj"], P["p1"], P["p0"]
    rho_c, es1, es0 = _fold_constants(P, modes)
    rho_c = rho_c[None, :]
    Em = ALPHA * ext[:, None] + es1[None, :] * s + es0[None, :]
    Pj = {j: p1[j][None, :] * s + p0[j][None, :]
          for j, m in enumerate(modes) if m == "max"}
    e = s.copy()
    K = np.clip(s, -KCLIP, KCLIP) if clip_on else s.copy()
    for t in range(1, STEPS + 1):
        u = e + gamma * K
        if t == STEPS:
            break
        acc = rho_c * u + Em
        for j in Pj:
            acc = acc + cj[j][None, :] * np.maximum(u, Pj[j])
        e = acc
        K1 = K + e
        K = np.clip(K1, -KCLIP, KCLIP) if clip_on else K1
    meas = kp * u
    return meas.mean(axis=1)


# ---------------------------------------------------------------------------
# device program
# ---------------------------------------------------------------------------

def build_program(P, modes, subs_per_chunk, n_chunks, clip_on=True):
    import concourse.bacc as bacc
    import concourse.mybir as mybir
    from concourse.tile import TileContext

    fp32 = mybir.dt.float32
    fp16 = mybir.dt.float16
    AF = mybir.ActivationFunctionType
    OP = mybir.AluOpType

    rows_pc = n_chunks * subs_per_chunk * SUB
    FD = 512 * subs_per_chunk

    uniform = (np.ptp(P["kp"]) == 0 and np.ptp(P["gamma"]) == 0
               and np.ptp(P["delta"]) == 0)

    eta, cj = P["eta"], P["cj"]
    rho_c, es1, es0 = _fold_constants(P, modes)
    max_branches = [j for j, m in enumerate(modes) if m == "max"]
    es1_zero = bool(np.all(es1 == 0.0))

    # ---- packed constants ----------------------------------------------
    p_idx = np.arange(128)
    c_of_p = p_idx % 8
    cols = {}
    cf_list = []

    def addcol(name, vec):
        cols[name] = len(cf_list)
        cf_list.append(np.asarray(vec, np.float64))

    addcol("gamma", P["gamma"][c_of_p])
    addcol("rho", rho_c[c_of_p])
    addcol("b1", P["b1"][p_idx % 16])
    addcol("b2", P["b2"][p_idx % 8])
    addcol("b3", P["b3"][p_idx % 8])
    addcol("es1", es1[c_of_p])
    addcol("es0", es0[c_of_p])
    for j in max_branches:
        addcol(f"p1_{j}", P["p1"][j][c_of_p])
        addcol(f"p0_{j}", P["p0"][j][c_of_p])
        addcol(f"c_{j}", cj[j][c_of_p])
    if P["kd_nonzero"]:
        addcol("delta", P["delta"][c_of_p])
    nscal = len(cf_list)
    ident_off = nscal
    cf32_np = np.zeros((128, nscal + 128), np.float32)
    cf32_np[:, :nscal] = np.stack(cf_list, axis=1)
    cf32_np[:, ident_off:] = np.eye(128, dtype=np.float32)

    # block-diagonal stationary weights (see MLP section)
    o_w1, o_w2, o_w3, o_ext, o_ones = 0, 32, 64, 96, 128
    w16 = np.zeros((128, 144), np.float16)
    for p in range(128):
        tau, f32_ = p // 32, p % 32
        # W1dd: K=64 window rows 32*tau'+f, cols 16*w+i -> W1[i,f] iff tau'%2==w
        for i in range(16):
            w16[p, o_w1 + 16 * (tau % 2) + i] = np.float16(P["W1"][i, f32_])
        # W2qd: K=64 window rows (p%64) = 16*v+f, cols 8*v+i2 -> W2[i2,f]
        v2, f16_ = (p % 64) // 16, p % 16
        for i in range(8):
            w16[p, o_w2 + 8 * v2 + i] = np.float16(P["W2"][i, f16_])
        # W3qd: K=32 window rows (p%32) = 8*v+f, cols 8*v+c -> W3[c,f]
        v3, f8_ = (p % 32) // 8, p % 8
        for cc in range(8):
            w16[p, o_w3 + 8 * v3 + cc] = np.float16(P["W3"][cc, f8_])
        # extq: K=128 rows 32*tau+f, cols 8*v+c -> ALPHA iff f==1, v==tau
        if f32_ == 1:
            for cc in range(8):
                w16[p, o_ext + 8 * tau + cc] = np.float16(ALPHA)
        # onesW: K=128 rows 8*j+c, col j -> kp_c/8
        w16[p, o_ones + p // 8] = np.float16(P["kp"][p % 8] / 8.0)

    nc = bacc.Bacc("TRN2", target_bir_lowering=False, debug=False,
                   num_devices=N_CORES)

    x_d = nc.dram_tensor("x", [rows_pc, D_IN], fp32, kind="ExternalInput")
    cf32_d = nc.dram_tensor("cf32", list(cf32_np.shape), fp32,
                            kind="ExternalInput")
    cf16_d = nc.dram_tensor("cf16", list(w16.shape), fp16,
                            kind="ExternalInput")
    out_d = nc.dram_tensor("out", [rows_pc], fp32, kind="ExternalOutput")

    x_ap = x_d.ap()
    out_r = out_d.ap().rearrange(
        "(ch sc a d tau p) -> ch a tau sc d p",
        ch=n_chunks, sc=subs_per_chunk, a=4, d=4, tau=4, p=128)

    ENG = {"v": nc.vector, "p": nc.gpsimd, "s": nc.scalar}

    with TileContext(nc) as tc:
        with tc.tile_pool(name="const", bufs=1) as constp, \
             tc.tile_pool(name="mlp", bufs=3) as mlpp, \
             tc.tile_pool(name="state", bufs=2) as statep, \
             tc.tile_pool(name="outp", bufs=2) as outpool, \
             tc.tile_pool(name="ptrans", bufs=2, space="PSUM") as ptrans, \
             tc.tile_pool(name="pmm", bufs=1, space="PSUM") as pmm, \
             tc.tile_pool(name="pout", bufs=1, space="PSUM") as pout:

            cf32 = constp.tile([128, cf32_np.shape[1]], fp32)
            cf16 = constp.tile([128, w16.shape[1]], fp16)
            nc.sync.dma_start(out=cf32[:], in_=cf32_d.ap())
            nc.sync.dma_start(out=cf16[:], in_=cf16_d.ap())

            ident = cf32[:, ident_off:ident_off + 128]
            W1dd = cf16[:, o_w1:o_w1 + 32]
            W2qd = cf16[:, o_w2:o_w2 + 32]
            W3qd = cf16[:, o_w3:o_w3 + 32]
            extq = cf16[:, o_ext:o_ext + 32]
            onesW = cf16[:, o_ones:o_ones + 16]

        # scalar operands: python float when uniform, else (128,1) AP
            def sc_(name, vec):
                if uniform:
                    return float(np.asarray(vec).reshape(-1)[0])
                return cf32[:, cols[name]:cols[name] + 1]

            gammaS = sc_("gamma", P["gamma"])
            rhoS = sc_("rho", rho_c)
            cS = {j: sc_(f"c_{j}", cj[j]) for j in max_branches}
            p1S = {j: sc_(f"p1_{j}", P["p1"][j]) for j in max_branches}
            p0S = {j: sc_(f"p0_{j}", P["p0"][j]) for j in max_branches}
            es1S = sc_("es1", es1)
            es0S = sc_("es0", es0)
            deltaS = sc_("delta", P["delta"]) if P["kd_nonzero"] else None
            b1A = cf32[:, cols["b1"]:cols["b1"] + 1]
            b2A = cf32[:, cols["b2"]:cols["b2"] + 1]
            b3A = cf32[:, cols["b3"]:cols["b3"] + 1]

            # per-chunk tiles (explicit tags; bufs=1 pool)
            CH = []
            for ch in range(n_chunks):
                T = {}
                for nm in ("s16", "Em", "K", "eB", "u", "rt"):
                    T[nm] = statep.tile([128, FD], fp16, tag=f"{nm}{ch}",
                                        name=f"{nm}{ch}", bufs=1)
                T["P"] = {j: statep.tile([128, FD], fp16, tag=f"P{j}_{ch}",
                                         name=f"P{j}_{ch}", bufs=1)
                          for j in max_branches}
                T["v"] = {j: statep.tile([128, FD], fp16, tag=f"v{j}_{ch}",
                                         name=f"v{j}_{ch}", bufs=1)
                          for j in max_branches}
                T["q"] = {j: statep.tile([128, FD], fp16, tag=f"q{j}_{ch}",
                                         name=f"q{j}_{ch}", bufs=1)
                          for j in max_branches[1:]}
                if not es1_zero:
                    T["at"] = statep.tile([128, FD], fp16, tag=f"at{ch}",
                                          name=f"at{ch}", bufs=1)
                if P["kd_nonzero"]:
                    T["epv"] = statep.tile([128, FD], fp16, tag=f"epv{ch}",
                                           name=f"epv{ch}", bufs=1)
                    T["t2"] = statep.tile([128, FD], fp16, tag=f"t2{ch}",
                                          name=f"t2{ch}", bufs=1)
                CH.append(T)

            # ---------------- phase A: MLP + precompute ------------------
            for ch in range(n_chunks):
                T = CH[ch]
                s16, Em, Kt = T["s16"], T["Em"], T["K"]
                Pt = T["P"]
                r0 = ch * subs_per_chunk * SUB
                for sc in range(subs_per_chunk):
                    rs = r0 + sc * SUB
                    co = 512 * sc
                    xa = mlpp.tile([128, 2048], fp32, tag="xa")
                    nc.sync.dma_start(
                        out=xa[:],
                        in_=x_ap[rs:rs + SUB, :].rearrange(
                            "(t p) f -> p t f", p=128))

                    xT = mlpp.tile([128, 2048], fp16, tag="xT")
                    for g in range(4):
                        tp = ptrans.tile([128, 512], fp32, tag="tp")
                        for k in range(4):
                            D = 4 * g + k
                            nc.tensor.transpose(
                                out=tp[:, 128 * k:128 * k + 128],
                                in_=xa[:, 128 * D:128 * D + 128],
                                identity=ident)
                        nc.scalar.activation(
                            out=xT[:, 512 * g:512 * g + 512], in_=tp[:],
                            func=AF.Copy)

                    # L1: q = 2a + taup; out (32,512) at psum tile q//4,
                    # position 32*(q%4); rows 16*w+i with tau = 2*taup+w.
                    h1t = []
                    hps = [pmm.tile([128, 512], fp32, tag="hpA", name="hpA"),
                           pmm.tile([128, 512], fp32, tag="hpB", name="hpB")]
                    for a in range(4):
                        for taup in range(2):
                            q = 2 * a + taup
                            hp = hps[q // 4]
                            pos = 32 * (q % 4)
                            nc.tensor.matmul(
                                out=hp[pos:pos + 32, :],
                                lhsT=W1dd[64 * taup:64 * taup + 64, :],
                                rhs=xT[64 * taup:64 * taup + 64,
                                       512 * a:512 * a + 512],
                                tile_position=(64 * taup, pos))
                    for half in range(2):
                        h1 = mlpp.tile([128, 512], fp16, tag=f"h1_{half}",
                                       name=f"h1_{half}")
                        nc.scalar.activation(out=h1[:], in_=hps[half][:],
                                             func=AF.Relu, bias=b1A)
                        h1t.append(h1)

                    # L2: window (t,h) covers a = 2t+h (4 j's dense);
                    # out (32,512) at position 32a.
                    hp2 = pmm.tile([128, 512], fp32, tag="hp2")
                    for t_ in range(2):
                        for h_ in range(2):
                            a = 2 * t_ + h_
                            nc.tensor.matmul(
                                out=hp2[32 * a:32 * a + 32, :],
                                lhsT=W2qd[64 * h_:64 * h_ + 64, :],
                                rhs=h1t[t_][64 * h_:64 * h_ + 64, :],
                                tile_position=(64 * h_, 32 * a))
                    h2 = mlpp.tile([128, 512], fp16, tag="h2")
                    nc.scalar.activation(out=h2[:], in_=hp2[:],
                                         func=AF.Relu, bias=b2A)

                    # L3: dense in partitions 8j+c already.
                    hp3 = pmm.tile([128, 512], fp32, tag="hp3")
                    for m in range(4):
                        nc.tensor.matmul(
                            out=hp3[32 * m:32 * m + 32, :],
                            lhsT=W3qd[32 * m:32 * m + 32, :],
                            rhs=h2[32 * m:32 * m + 32, :],
                            tile_position=(32 * m, 32 * m))
                    nc.scalar.activation(out=s16[:, co:co + 512], in_=hp3[:],
                                         func=AF.Identity, bias=b3A)

                    # EXT: alpha * x[:,1] replicated over c, dense 8j+c.
                    hpe = pmm.tile([128, 512], fp32, tag="hpe")
                    for a in range(4):
                        nc.tensor.matmul(
                            out=hpe[32 * a:32 * a + 32, :],
                            lhsT=extq[:, :],
                            rhs=xT[:, 512 * a:512 * a + 512],
                            tile_position=(0, 32 * a))
                    if es1_zero:
                        nc.scalar.activation(out=Em[:, co:co + 512],
                                             in_=hpe[:], func=AF.Identity,
                                             bias=es0S if not uniform
                                             else float(es0[0]))
                    else:
                        nc.scalar.activation(out=Em[:, co:co + 512],
                                             in_=hpe[:], func=AF.Identity,
                                             bias=0.0)

                if not es1_zero:
                    nc.vector.tensor_scalar(out=T["at"][:], in0=s16[:],
                                            scalar1=es1S, scalar2=es0S,
                                            op0=OP.mult, op1=OP.add)
                    nc.vector.tensor_add(out=Em[:], in0=Em[:], in1=T["at"][:])
                for j in max_branches:
                    nc.vector.tensor_scalar(out=Pt[j][:], in0=s16[:],
                                            scalar1=p1S[j], scalar2=p0S[j],
                                            op0=OP.mult, op1=OP.add)
                if clip_on:
                    nc.vector.tensor_scalar(out=Kt[:], in0=s16[:],
                                            scalar1=KCLIP, scalar2=-KCLIP,
                                            op0=OP.min, op1=OP.max)
                else:
                    nc.vector.tensor_copy(out=Kt[:], in_=s16[:])
                if P["kd_nonzero"]:
                    nc.vector.memset(T["epv"][:], 0.0)

            # ---------------- phase B: scan, step-outer ------------------
            # chunks interleave so cross-engine latency (ACT, DMA-accum)
            # hides behind other chunks' DVE work
            for t in range(1, STEPS + 1):
                for ch in range(n_chunks):
                    T = CH[ch]
                    Kt, ut, rt, Em = T["K"], T["u"], T["rt"], T["Em"]
                    ebufs = [T["s16"], T["eB"]]
                    e_in = ebufs[(t - 1) % 2]
                    e_out = ebufs[t % 2]
                    if t > 1:
                        nc.vector.tensor_add(out=Kt[:], in0=Kt[:],
                                             in1=e_in[:])
                        if clip_on:
                            nc.vector.tensor_scalar(
                                out=Kt[:], in0=Kt[:], scalar1=KCLIP,
                                scalar2=-KCLIP, op0=OP.min, op1=OP.max)
                    # u := gamma*K (ACT), then u += e via DMA CCE
                    nc.scalar.activation(out=ut[:], in_=Kt[:], func=AF.Copy,
                                         scale=gammaS)
                    nc.gpsimd.dma_start(out=ut[:], in_=e_in[:],
                                        accum_op=OP.add)
                    if P["kd_nonzero"]:
                        nc.vector.tensor_sub(out=T["t2"][:], in0=e_in[:],
                                             in1=T["epv"][:])
                        nc.vector.scalar_tensor_tensor(
                            out=ut[:], in0=T["t2"][:], scalar=deltaS,
                            in1=ut[:], op0=OP.mult, op1=OP.add)
                        nc.vector.tensor_copy(out=T["epv"][:], in_=e_in[:])
                    if t == STEPS:
                        continue
                    first = max_branches[0] if max_branches else None
                    for j in max_branches:
                        nc.vector.tensor_max(out=T["v"][j][:], in0=ut[:],
                                             in1=T["P"][j][:])
                        if j == first:
                            nc.vector.tensor_scalar_mul(
                                out=e_out[:], in0=T["v"][j][:], scalar1=cS[j])
                        else:
                            nc.vector.tensor_scalar_mul(
                                out=T["q"][j][:], in0=T["v"][j][:],
                                scalar1=cS[j])
                            nc.vector.tensor_add(out=e_out[:], in0=e_out[:],
                                                 in1=T["q"][j][:])
                    # rt := rho*u (ACT)
                    nc.scalar.activation(out=rt[:], in_=ut[:], func=AF.Copy,
                                         scale=rhoS)
                    if first is None:
                        nc.vector.tensor_add(out=e_out[:], in0=rt[:],
                                             in1=Em[:])
                    else:
                        nc.vector.tensor_add(out=e_out[:], in0=e_out[:],
                                             in1=rt[:])
                        nc.vector.tensor_add(out=e_out[:], in0=e_out[:],
                                             in1=Em[:])

            # ---------------- phase C: reduce + store --------------------
            for ch in range(n_chunks):
                ut = CH[ch]["u"]
                outS = outpool.tile([16, FD], fp32, tag="outS")
                for z in range(subs_per_chunk):
                    ro = pout.tile([16, 512], fp32, tag="ro")
                    nc.tensor.matmul(out=ro[:], lhsT=onesW[:],
                                     rhs=ut[:, 512 * z:512 * z + 512])
                    nc.scalar.activation(out=outS[:, 512 * z:512 * z + 512],
                                         in_=ro[:], func=AF.Copy)
                for a in range(4):
                    for z in range(subs_per_chunk):
                        nc.sync.dma_start(
                            out=out_r[ch, a][:, z],
                            in_=outS[4 * a:4 * a + 4,
                                     512 * z:512 * z + 512].rearrange(
                                "t (d p) -> t d p", d=4, p=128))

    nc.compile()
    return nc, cf32_np, w16


# ---------------------------------------------------------------------------
# entry point
# ---------------------------------------------------------------------------

_CACHE = {}


def _get_program(P, modes, subs_per_chunk, n_chunks, clip_on=True):
    key = (tuple(modes), subs_per_chunk, n_chunks, clip_on,
           P["kd_nonzero"], tuple(np.asarray(P["kp"]).tolist()))
    if key not in _CACHE:
        _CACHE[key] = build_program(P, modes, subs_per_chunk, n_chunks,
                                    clip_on)
    return _CACHE[key]


LAST_RESULT = None


def kernel(**inputs):
    import os
    from concourse.bass_utils import run_bass_kernel_spmd

    x = np.ascontiguousarray(np.asarray(inputs["x"], np.float32))
    B = x.shape[0]
    assert B % N_CORES == 0
    rows_pc = B // N_CORES

    P = _derive(inputs)
    modes = _pick_branch_modes(inputs, P)

    # drop the integral clip when its numerical impact is negligible
    xs = x[:: max(1, x.shape[0] // 65536)][:65536].astype(np.float64)
    base = host_restructured(xs, P, modes, clip_on=True)
    nocl = host_restructured(xs, P, modes, clip_on=False)
    clip_on = bool(np.linalg.norm(nocl - base) > 2e-3 * np.linalg.norm(base))

    assert rows_pc % SUB == 0
    n_subs = rows_pc // SUB
    subs_per_chunk = 4 if n_subs % 4 == 0 else 1
    n_chunks = n_subs // subs_per_chunk

    nc, cf32_np, w16 = _get_program(P, modes, subs_per_chunk, n_chunks,
                                    clip_on)

    in_maps = []
    for k in range(N_CORES):
        in_maps.append({
            "x": x[k * rows_pc:(k + 1) * rows_pc],
            "cf32": cf32_np,
            "cf16": w16,
        })
    trace = bool(int(os.environ.get("KERNEL_TRACE", "0")))
    res = run_bass_kernel_spmd(nc, in_maps, core_ids=list(range(N_CORES)),
                               trace=trace)
    global LAST_RESULT
    LAST_RESULT = res
    out = np.concatenate([np.asarray(res.results[k]["out"]).reshape(-1)
                          for k in range(N_CORES)])
    return out.astype(np.float32)


# revision 32
# speedup vs baseline: 1.1345x; 1.1345x over previous
"""Trainium2 Bass kernel for nn_AdaptivePIDNetworkControllerV2.

Self-contained: kernel(**inputs) -> np.ndarray (B,) float32.

Algorithm
---------
Reference, per batch row b:
  ext = x[b,1];  s_c = MLP(x[b]) (32->16->8->C; relu, relu, linear)
  25-step PID scan per controller c over state
  (meas, dyn_sp, integral, prev_err); output mean_c(meas_25).

With Kd == 0 the scan reduces exactly to a 2-state recurrence in
(e = err, K = integral/DT), with u = e + gamma*K (meas = Kp*u):
  e'  = E''' + rho*u + sum_j c_j * max(u, P_j)
  K'  = clip(K + e', +-50)
where P_j = p1_j*s + p0_j (affine in s), E''' = ALPHA*ext + const, and
rho/c_j/p*_j are scalars derived from the dynamic-setpoint network
weights (relu(z_j) rewritten via max identities; signs folded).
Branches whose relu never clamps on a host subsample are folded into
rho exactly ("linear_u").

Layout: batch sharded 8 ways (pure data parallel).  Per core, rows go
through chunks of SUBS_PER_CHUNK*8192; the scan runs on fp16 tiles
(128, 512*SUBS_PER_CHUNK) with partition = 8*j + c and batch row
  row = chunk_base + 8192*sc + 2048*(j//4) + 512*d + 128*(j%4) + p
for column index 512*sc + 128*d + p.  The MLP runs feature-on-partition
via PE transposes and partition-stacked small matmuls; ext is extracted
with an extra "selector" matmul; the final mean over c is a matmul with
a block-diagonal (Kp_c/8) matrix.
"""

import numpy as np

B_TOTAL = 1048576
D_IN = 32
C = 8
STEPS = 25
ALPHA = 0.5
DT = 0.1
KCLIP = 5.0 / DT          # integral clip bound in K units

N_CORES = 8
SUB = 8192                # rows per MLP sub-chunk (fixed internal shape)



# ---------------------------------------------------------------------------
# host-side derivation
# ---------------------------------------------------------------------------

def _derive(inputs):
    f = lambda k: np.asarray(inputs[k], np.float64)
    P = dict(
        W1=f("W1"), b1=f("b1"), W2=f("W2"), b2=f("b2"),
        W3=f("W3"), b3=f("b3"),
    )
    Wd1, bd1 = f("Wd1"), f("bd1")
    Wd2, bd2 = f("Wd2").reshape(-1), f("bd2")
    Kp, Ki, Kd = f("Kp"), f("Ki"), f("Kd")

    P["kp"] = Kp
    P["gamma"] = Ki * DT / Kp
    P["delta"] = Kd / (DT * Kp)
    a_j = Wd1[:, 0]
    braw = Wd1[:, 1]
    w_j = (1 - ALPHA) * Wd2
    P["w0"] = (1 - ALPHA) * bd2[0]
    beta = braw[:, None] * Kp[None, :]           # (3, C)
    P["eta"] = np.sign(beta)
    P["cj"] = w_j[:, None] * np.abs(beta)        # (3, C)
    P["p1"] = -a_j[:, None] / beta               # (3, C)
    P["p0"] = -bd1[:, None] / beta               # (3, C)
    P["kd_nonzero"] = bool(np.any(np.abs(Kd) > 0))
    return P


def _host_mlp(x, P):
    h = np.maximum(x @ P["W1"].T + P["b1"], 0)
    h = np.maximum(h @ P["W2"].T + P["b2"], 0)
    return h @ P["W3"].T + P["b3"]


def _pick_branch_modes(inputs, P):
    """Decide per branch j: 'max' (keep), 'linear_u' (max(u,P)==u always),
    or 'linear_p' (==P always), from a host subsample of the recurrence."""
    if P["kd_nonzero"]:
        return ["max", "max", "max"]
    x = np.asarray(inputs["x"], np.float64)
    n = min(65536, x.shape[0])
    step = max(1, x.shape[0] // n)
    xs = x[::step][:n]
    s = _host_mlp(xs, P)
    ext = xs[:, 1]
    gamma = P["gamma"][None, :]
    kp = P["kp"][None, :]
    eta, cj, p1, p0 = P["eta"], P["cj"], P["p1"], P["p0"]
    Pj = p1[:, None, :] * s[None] + p0[:, None, :]
    rho = -kp - np.where(eta < 0, cj, 0.0).sum(0)[None, :]
    Em = ALPHA * ext[:, None] + P["w0"] \
        - (np.where(eta > 0, cj, 0.0)[:, None, :] * Pj).sum(0)
    def run(branch_fn):
        e = s.copy()
        K = np.clip(s, -KCLIP, KCLIP)
        for t in range(1, STEPS + 1):
            u = e + gamma * K
            if t == STEPS:
                break
            acc = rho * u + Em
            for j in range(3):
                acc = acc + cj[j][None, :] * branch_fn(j, u)
            e = acc
            K = np.clip(K + e, -KCLIP, KCLIP)
        return (kp * u).mean(axis=1)

    base = run(lambda j, u: np.maximum(u, Pj[j]))
    nrm = np.linalg.norm(base)
    modes = []
    for j in range(3):
        def lin_u(jj, u, j=j):
            return u if jj == j else np.maximum(u, Pj[jj])
        def lin_p(jj, u, j=j):
            return Pj[jj] if jj == j else np.maximum(u, Pj[jj])
        if np.linalg.norm(run(lin_u) - base) < 1e-4 * nrm:
            modes.append("linear_u")
        elif np.linalg.norm(run(lin_p) - base) < 1e-4 * nrm:
            modes.append("linear_p")
        else:
            modes.append("max")
    return modes


def _fold_constants(P, modes):
    """Per-branch contribution of c_j*relu-term to (rho, es1, es0), using
    hd_j = |beta_j|*relu(eta_j*(u - P_j)) identities:
      eta>0: c*relu(u-P)  = c*max(u,P) - c*P   (max mode)
                          = c*u - c*P          (linear_u)
                          = 0                  (linear_p)
      eta<0: c*relu(P-u)  = c*max(u,P) - c*u   (max mode)
                          = 0                  (linear_u)
                          = c*P - c*u          (linear_p)
    "max" mode additionally emits the c_j*max(u,P_j) op on device."""
    cj, eta, p1, p0 = P["cj"], P["eta"], P["p1"], P["p0"]
    rho = -P["kp"].copy()
    es1 = np.zeros(C)
    es0 = np.full(C, P["w0"])
    for j, m in enumerate(modes):
        pos = eta[j, 0] > 0
        if m == "max":
            if pos:
                es1 = es1 - cj[j] * p1[j]
                es0 = es0 - cj[j] * p0[j]
            else:
                rho = rho - cj[j]
        elif m == "linear_u":
            if pos:
                rho = rho + cj[j]
                es1 = es1 - cj[j] * p1[j]
                es0 = es0 - cj[j] * p0[j]
        elif m == "linear_p":
            if not pos:
                rho = rho - cj[j]
                es1 = es1 + cj[j] * p1[j]
                es0 = es0 + cj[j] * p0[j]
    return rho, es1, es0


def host_restructured(x, P, modes, clip_on=True):
    """Full-precision host implementation of exactly what the device
    computes (minus fp16 rounding); used for self-checks in test.py."""
    s = _host_mlp(x, P)
    ext = x[:, 1]
    gamma = P["gamma"][None, :]
    kp = P["kp"][None, :]
    eta, cj, p1, p0 = P["eta"], P["cj"], P["p1"], P["p0"]
    rho_c, es1, es0 = _fold_constants(P, modes)
    rho_c = rho_c[None, :]
    Em = ALPHA * ext[:, None] + es1[None, :] * s + es0[None, :]
    Pj = {j: p1[j][None, :] * s + p0[j][None, :]
          for j, m in enumerate(modes) if m == "max"}
    e = s.copy()
    K = np.clip(s, -KCLIP, KCLIP) if clip_on else s.copy()
    for t in range(1, STEPS + 1):
        u = e + gamma * K
        if t == STEPS:
            break
        acc = rho_c * u + Em
        for j in Pj:
            acc = acc + cj[j][None, :] * np.maximum(u, Pj[j])
        e = acc
        K1 = K + e
        K = np.clip(K1, -KCLIP, KCLIP) if clip_on else K1
    meas = kp * u
    return meas.mean(axis=1)


# ---------------------------------------------------------------------------
# device program
# ---------------------------------------------------------------------------

def build_program(P, modes, subs_per_chunk, n_chunks, clip_on=True):
    import concourse.bacc as bacc
    import concourse.mybir as mybir
    from concourse.tile import TileContext

    fp32 = mybir.dt.float32
    fp16 = mybir.dt.float16
    AF = mybir.ActivationFunctionType
    OP = mybir.AluOpType

    rows_pc = n_chunks * subs_per_chunk * SUB
    FD = 512 * subs_per_chunk

    uniform = (np.ptp(P["kp"]) == 0 and np.ptp(P["gamma"]) == 0
               and np.ptp(P["delta"]) == 0)

    eta, cj = P["eta"], P["cj"]
    rho_c, es1, es0 = _fold_constants(P, modes)
    max_branches = [j for j, m in enumerate(modes) if m == "max"]
    es1_zero = bool(np.all(es1 == 0.0))

    # ---- packed constants ----------------------------------------------
    p_idx = np.arange(128)
    c_of_p = p_idx % 8
    cols = {}
    cf_list = []

    def addcol(name, vec):
        cols[name] = len(cf_list)
        cf_list.append(np.asarray(vec, np.float64))

    addcol("gamma", P["gamma"][c_of_p])
    addcol("rho", rho_c[c_of_p])
    addcol("b1", P["b1"][p_idx % 16])
    addcol("b2", P["b2"][p_idx % 8])
    addcol("b3", P["b3"][p_idx % 8])
    addcol("es1", es1[c_of_p])
    addcol("es0", es0[c_of_p])
    for j in max_branches:
        addcol(f"p1_{j}", P["p1"][j][c_of_p])
        addcol(f"p0_{j}", P["p0"][j][c_of_p])
        addcol(f"c_{j}", cj[j][c_of_p])
    if P["kd_nonzero"]:
        addcol("delta", P["delta"][c_of_p])
    nscal = len(cf_list)
    ident_off = nscal
    cf32_np = np.zeros((128, nscal + 128), np.float32)
    cf32_np[:, :nscal] = np.stack(cf_list, axis=1)
    cf32_np[:, ident_off:] = np.eye(128, dtype=np.float32)

    # block-diagonal stationary weights (see MLP section)
    o_w1, o_w2, o_w3, o_ext, o_ones = 0, 32, 64, 96, 128
    w16 = np.zeros((128, 144), np.float16)
    for p in range(128):
        tau, f32_ = p // 32, p % 32
        # W1dd: K=64 window rows 32*tau'+f, cols 16*w+i -> W1[i,f] iff tau'%2==w
        for i in range(16):
            w16[p, o_w1 + 16 * (tau % 2) + i] = np.float16(P["W1"][i, f32_])
        # W2qd: K=64 window rows (p%64) = 16*v+f, cols 8*v+i2 -> W2[i2,f]
        v2, f16_ = (p % 64) // 16, p % 16
        for i in range(8):
            w16[p, o_w2 + 8 * v2 + i] = np.float16(P["W2"][i, f16_])
        # W3qd: K=32 window rows (p%32) = 8*v+f, cols 8*v+c -> W3[c,f]
        v3, f8_ = (p % 32) // 8, p % 8
        for cc in range(8):
            w16[p, o_w3 + 8 * v3 + cc] = np.float16(P["W3"][cc, f8_])
        # extq: K=128 rows 32*tau+f, cols 8*v+c -> ALPHA iff f==1, v==tau
        if f32_ == 1:
            for cc in range(8):
                w16[p, o_ext + 8 * tau + cc] = np.float16(ALPHA)
        # onesW: K=128 rows 8*j+c, col j -> kp_c/8
        w16[p, o_ones + p // 8] = np.float16(P["kp"][p % 8] / 8.0)

    nc = bacc.Bacc("TRN2", target_bir_lowering=False, debug=False,
                   num_devices=N_CORES)

    x_d = nc.dram_tensor("x", [rows_pc, D_IN], fp32, kind="ExternalInput")
    cf32_d = nc.dram_tensor("cf32", list(cf32_np.shape), fp32,
                            kind="ExternalInput")
    cf16_d = nc.dram_tensor("cf16", list(w16.shape), fp16,
                            kind="ExternalInput")
    out_d = nc.dram_tensor("out", [rows_pc], fp32, kind="ExternalOutput")

    x_ap = x_d.ap()
    out_r = out_d.ap().rearrange(
        "(ch sc a d tau p) -> ch a tau sc d p",
        ch=n_chunks, sc=subs_per_chunk, a=4, d=4, tau=4, p=128)

    ENG = {"v": nc.vector, "p": nc.gpsimd, "s": nc.scalar}

    with TileContext(nc) as tc:
        with tc.tile_pool(name="const", bufs=1) as constp, \
             tc.tile_pool(name="mlp", bufs=3) as mlpp, \
             tc.tile_pool(name="state", bufs=2) as statep, \
             tc.tile_pool(name="outp", bufs=2) as outpool, \
             tc.tile_pool(name="ptrans", bufs=2, space="PSUM") as ptrans, \
             tc.tile_pool(name="pmm", bufs=1, space="PSUM") as pmm, \
             tc.tile_pool(name="pout", bufs=1, space="PSUM") as pout:

            cf32 = constp.tile([128, cf32_np.shape[1]], fp32)
            cf16 = constp.tile([128, w16.shape[1]], fp16)
            nc.sync.dma_start(out=cf32[:], in_=cf32_d.ap())
            nc.sync.dma_start(out=cf16[:], in_=cf16_d.ap())

            ident = cf32[:, ident_off:ident_off + 128]
            W1dd = cf16[:, o_w1:o_w1 + 32]
            W2qd = cf16[:, o_w2:o_w2 + 32]
            W3qd = cf16[:, o_w3:o_w3 + 32]
            extq = cf16[:, o_ext:o_ext + 32]
            onesW = cf16[:, o_ones:o_ones + 16]

        # scalar operands: python float when uniform, else (128,1) AP
            def sc_(name, vec):
                if uniform:
                    return float(np.asarray(vec).reshape(-1)[0])
                return cf32[:, cols[name]:cols[name] + 1]

            gammaS = sc_("gamma", P["gamma"])
            rhoS = sc_("rho", rho_c)
            cS = {j: sc_(f"c_{j}", cj[j]) for j in max_branches}
            p1S = {j: sc_(f"p1_{j}", P["p1"][j]) for j in max_branches}
            p0S = {j: sc_(f"p0_{j}", P["p0"][j]) for j in max_branches}
            es1S = sc_("es1", es1)
            es0S = sc_("es0", es0)
            deltaS = sc_("delta", P["delta"]) if P["kd_nonzero"] else None
            b1A = cf32[:, cols["b1"]:cols["b1"] + 1]
            b2A = cf32[:, cols["b2"]:cols["b2"] + 1]
            b3A = cf32[:, cols["b3"]:cols["b3"] + 1]

            def evac(dst, srcp, bias=None, relu=False, on_dve=False):
                """psum->sbuf evacuation with optional per-partition bias
                and relu; routed to DVE (tensor_scalar) or ACT."""
                if on_dve:
                    if relu:
                        nc.vector.tensor_scalar(
                            out=dst, in0=srcp, scalar1=bias, scalar2=0.0,
                            op0=OP.add, op1=OP.max)
                    elif bias is not None:
                        nc.vector.tensor_scalar_add(out=dst, in0=srcp,
                                                    scalar1=bias)
                    else:
                        nc.vector.tensor_copy(out=dst, in_=srcp)
                else:
                    if relu:
                        nc.scalar.activation(out=dst, in_=srcp, func=AF.Relu,
                                             bias=bias)
                    elif bias is not None:
                        nc.scalar.activation(out=dst, in_=srcp,
                                             func=AF.Identity, bias=bias)
                    else:
                        nc.scalar.activation(out=dst, in_=srcp, func=AF.Copy)

            # per-chunk tiles (explicit tags; bufs=1 pool)
            CH = []
            for ch in range(n_chunks):
                T = {}
                for nm in ("s16", "Em", "K", "eB", "u", "rt"):
                    T[nm] = statep.tile([128, FD], fp16, tag=f"{nm}{ch}",
                                        name=f"{nm}{ch}", bufs=1)
                T["P"] = {j: statep.tile([128, FD], fp16, tag=f"P{j}_{ch}",
                                         name=f"P{j}_{ch}", bufs=1)
                          for j in max_branches}
                T["v"] = {j: statep.tile([128, FD], fp16, tag=f"v{j}_{ch}",
                                         name=f"v{j}_{ch}", bufs=1)
                          for j in max_branches}
                T["q"] = {j: statep.tile([128, FD], fp16, tag=f"q{j}_{ch}",
                                         name=f"q{j}_{ch}", bufs=1)
                          for j in max_branches[1:]}
                if not es1_zero:
                    T["at"] = statep.tile([128, FD], fp16, tag=f"at{ch}",
                                          name=f"at{ch}", bufs=1)
                if P["kd_nonzero"]:
                    T["epv"] = statep.tile([128, FD], fp16, tag=f"epv{ch}",
                                           name=f"epv{ch}", bufs=1)
                    T["t2"] = statep.tile([128, FD], fp16, tag=f"t2{ch}",
                                          name=f"t2{ch}", bufs=1)
                CH.append(T)

            # ---------------- phase A: MLP + precompute ------------------
            for ch in range(n_chunks):
                T = CH[ch]
                s16, Em, Kt = T["s16"], T["Em"], T["K"]
                Pt = T["P"]
                r0 = ch * subs_per_chunk * SUB
                for sc in range(subs_per_chunk):
                    rs = r0 + sc * SUB
                    co = 512 * sc
                    head_dve = (ch * subs_per_chunk + sc) < (
                        n_chunks * subs_per_chunk) // 2
                    xa = mlpp.tile([128, 2048], fp32, tag="xa")
                    nc.sync.dma_start(
                        out=xa[:],
                        in_=x_ap[rs:rs + SUB, :].rearrange(
                            "(t p) f -> p t f", p=128))

                    xT = mlpp.tile([128, 2048], fp16, tag="xT")
                    for g in range(4):
                        tp = ptrans.tile([128, 512], fp32, tag="tp")
                        for k in range(4):
                            D = 4 * g + k
                            nc.tensor.transpose(
                                out=tp[:, 128 * k:128 * k + 128],
                                in_=xa[:, 128 * D:128 * D + 128],
                                identity=ident)
                        evac(xT[:, 512 * g:512 * g + 512], tp[:],
                             on_dve=head_dve)

                    # L1: q = 2a + taup; out (32,512) at psum tile q//4,
                    # position 32*(q%4); rows 16*w+i with tau = 2*taup+w.
                    h1t = []
                    hps = [pmm.tile([128, 512], fp32, tag="hpA", name="hpA"),
                           pmm.tile([128, 512], fp32, tag="hpB", name="hpB")]
                    for a in range(4):
                        for taup in range(2):
                            q = 2 * a + taup
                            hp = hps[q // 4]
                            pos = 32 * (q % 4)
                            nc.tensor.matmul(
                                out=hp[pos:pos + 32, :],
                                lhsT=W1dd[64 * taup:64 * taup + 64, :],
                                rhs=xT[64 * taup:64 * taup + 64,
                                       512 * a:512 * a + 512],
                                tile_position=(64 * taup, pos))
                    for half in range(2):
                        h1 = mlpp.tile([128, 512], fp16, tag=f"h1_{half}",
                                       name=f"h1_{half}")
                        evac(h1[:], hps[half][:], bias=b1A, relu=True,
                             on_dve=head_dve)
                        h1t.append(h1)

                    # L2: window (t,h) covers a = 2t+h (4 j's dense);
                    # out (32,512) at position 32a.
                    hp2 = pmm.tile([128, 512], fp32, tag="hp2")
                    for t_ in range(2):
                        for h_ in range(2):
                            a = 2 * t_ + h_
                            nc.tensor.matmul(
                                out=hp2[32 * a:32 * a + 32, :],
                                lhsT=W2qd[64 * h_:64 * h_ + 64, :],
                                rhs=h1t[t_][64 * h_:64 * h_ + 64, :],
                                tile_position=(64 * h_, 32 * a))
                    h2 = mlpp.tile([128, 512], fp16, tag="h2")
                    evac(h2[:], hp2[:], bias=b2A, relu=True,
                         on_dve=head_dve)

                    # L3: dense in partitions 8j+c already.
                    hp3 = pmm.tile([128, 512], fp32, tag="hp3")
                    for m in range(4):
                        nc.tensor.matmul(
                            out=hp3[32 * m:32 * m + 32, :],
                            lhsT=W3qd[32 * m:32 * m + 32, :],
                            rhs=h2[32 * m:32 * m + 32, :],
                            tile_position=(32 * m, 32 * m))
                    evac(s16[:, co:co + 512], hp3[:], bias=b3A,
                         on_dve=head_dve)

                    # EXT: alpha * x[:,1] replicated over c, dense 8j+c.
                    hpe = pmm.tile([128, 512], fp32, tag="hpe")
                    for a in range(4):
                        nc.tensor.matmul(
                            out=hpe[32 * a:32 * a + 32, :],
                            lhsT=extq[:, :],
                            rhs=xT[:, 512 * a:512 * a + 512],
                            tile_position=(0, 32 * a))
                    if es1_zero:
                        evac(Em[:, co:co + 512], hpe[:],
                             bias=es0S if not uniform else float(es0[0]),
                             on_dve=head_dve)
                    else:
                        evac(Em[:, co:co + 512], hpe[:], on_dve=head_dve)

                if not es1_zero:
                    nc.vector.tensor_scalar(out=T["at"][:], in0=s16[:],
                                            scalar1=es1S, scalar2=es0S,
                                            op0=OP.mult, op1=OP.add)
                    nc.vector.tensor_add(out=Em[:], in0=Em[:], in1=T["at"][:])
                for j in max_branches:
                    nc.vector.tensor_scalar(out=Pt[j][:], in0=s16[:],
                                            scalar1=p1S[j], scalar2=p0S[j],
                                            op0=OP.mult, op1=OP.add)
                if clip_on:
                    nc.vector.tensor_scalar(out=Kt[:], in0=s16[:],
                                            scalar1=KCLIP, scalar2=-KCLIP,
                                            op0=OP.min, op1=OP.max)
                else:
                    nc.vector.tensor_copy(out=Kt[:], in_=s16[:])
                if P["kd_nonzero"]:
                    nc.vector.memset(T["epv"][:], 0.0)

            # ---------------- phase B: scan, step-outer ------------------
            # chunks interleave so cross-engine latency (ACT, DMA-accum)
            # hides behind other chunks' DVE work
            for t in range(1, STEPS + 1):
                for ch in range(n_chunks):
                    T = CH[ch]
                    Kt, ut, rt, Em = T["K"], T["u"], T["rt"], T["Em"]
                    ebufs = [T["s16"], T["eB"]]
                    e_in = ebufs[(t - 1) % 2]
                    e_out = ebufs[t % 2]
                    if t > 1:
                        nc.vector.tensor_add(out=Kt[:], in0=Kt[:],
                                             in1=e_in[:])
                        if clip_on:
                            nc.vector.tensor_scalar(
                                out=Kt[:], in0=Kt[:], scalar1=KCLIP,
                                scalar2=-KCLIP, op0=OP.min, op1=OP.max)
                    # u := gamma*K (ACT), then u += e via DMA CCE
                    nc.scalar.activation(out=ut[:], in_=Kt[:], func=AF.Copy,
                                         scale=gammaS)
                    nc.gpsimd.dma_start(out=ut[:], in_=e_in[:],
                                        accum_op=OP.add)
                    if P["kd_nonzero"]:
                        nc.vector.tensor_sub(out=T["t2"][:], in0=e_in[:],
                                             in1=T["epv"][:])
                        nc.vector.scalar_tensor_tensor(
                            out=ut[:], in0=T["t2"][:], scalar=deltaS,
                            in1=ut[:], op0=OP.mult, op1=OP.add)
                        nc.vector.tensor_copy(out=T["epv"][:], in_=e_in[:])
                    if t == STEPS:
                        continue
                    first = max_branches[0] if max_branches else None
                    for j in max_branches:
                        nc.vector.tensor_max(out=T["v"][j][:], in0=ut[:],
                                             in1=T["P"][j][:])
                        if j == first:
                            nc.vector.tensor_scalar_mul(
                                out=e_out[:], in0=T["v"][j][:], scalar1=cS[j])
                        else:
                            nc.vector.tensor_scalar_mul(
                                out=T["q"][j][:], in0=T["v"][j][:],
                                scalar1=cS[j])
                            nc.vector.tensor_add(out=e_out[:], in0=e_out[:],
                                                 in1=T["q"][j][:])
                    # rt := rho*u (ACT)
                    nc.scalar.activation(out=rt[:], in_=ut[:], func=AF.Copy,
                                         scale=rhoS)
                    if first is None:
                        nc.vector.tensor_add(out=e_out[:], in0=rt[:],
                                             in1=Em[:])
                    else:
                        nc.vector.tensor_add(out=e_out[:], in0=e_out[:],
                                             in1=rt[:])
                        nc.vector.tensor_add(out=e_out[:], in0=e_out[:],
                                             in1=Em[:])

            # ---------------- phase C: reduce + store --------------------
            for ch in range(n_chunks):
                ut = CH[ch]["u"]
                outS = outpool.tile([16, FD], fp32, tag="outS")
                for z in range(subs_per_chunk):
                    ro = pout.tile([16, 512], fp32, tag="ro")
                    nc.tensor.matmul(out=ro[:], lhsT=onesW[:],
                                     rhs=ut[:, 512 * z:512 * z + 512])
                    nc.scalar.activation(out=outS[:, 512 * z:512 * z + 512],
                                         in_=ro[:], func=AF.Copy)
                for a in range(4):
                    for z in range(subs_per_chunk):
                        nc.sync.dma_start(
                            out=out_r[ch, a][:, z],
                            in_=outS[4 * a:4 * a + 4,
                                     512 * z:512 * z + 512].rearrange(
                                "t (d p) -> t d p", d=4, p=128))

    nc.compile()
    return nc, cf32_np, w16


# ---------------------------------------------------------------------------
# entry point
# ---------------------------------------------------------------------------

_CACHE = {}


def _get_program(P, modes, subs_per_chunk, n_chunks, clip_on=True):
    key = (tuple(modes), subs_per_chunk, n_chunks, clip_on,
           P["kd_nonzero"], tuple(np.asarray(P["kp"]).tolist()))
    if key not in _CACHE:
        _CACHE[key] = build_program(P, modes, subs_per_chunk, n_chunks,
                                    clip_on)
    return _CACHE[key]


LAST_RESULT = None


def kernel(**inputs):
    import os
    from concourse.bass_utils import run_bass_kernel_spmd

    x = np.ascontiguousarray(np.asarray(inputs["x"], np.float32))
    B = x.shape[0]
    assert B % N_CORES == 0
    rows_pc = B // N_CORES

    P = _derive(inputs)
    modes = _pick_branch_modes(inputs, P)

    # drop the integral clip when its numerical impact is negligible
    xs = x[:: max(1, x.shape[0] // 65536)][:65536].astype(np.float64)
    base = host_restructured(xs, P, modes, clip_on=True)
    nocl = host_restructured(xs, P, modes, clip_on=False)
    clip_on = bool(np.linalg.norm(nocl - base) > 2e-3 * np.linalg.norm(base))

    assert rows_pc % SUB == 0
    n_subs = rows_pc // SUB
    subs_per_chunk = 4 if n_subs % 4 == 0 else 1
    n_chunks = n_subs // subs_per_chunk

    nc, cf32_np, w16 = _get_program(P, modes, subs_per_chunk, n_chunks,
                                    clip_on)

    in_maps = []
    for k in range(N_CORES):
        in_maps.append({
            "x": x[k * rows_pc:(k + 1) * rows_pc],
            "cf32": cf32_np,
            "cf16": w16,
        })
    trace = bool(int(os.environ.get("KERNEL_TRACE", "0")))
    res = run_bass_kernel_spmd(nc, in_maps, core_ids=list(range(N_CORES)),
                               trace=trace)
    global LAST_RESULT
    LAST_RESULT = res
    out = np.concatenate([np.asarray(res.results[k]["out"]).reshape(-1)
                          for k in range(N_CORES)])
    return out.astype(np.float32)
